# revision 7
# baseline (speedup 1.0000x reference)
"""Trainium2 Bass kernel for nn_CustomizeLSTMCell.

reference:
    pre = w_in_input @ s_in + w_out_input @ s_out + u_in_input @ h_in + u_out_input @ h_out
    g = sigmoid(pre)
    cell_state = g * last_c + g * g          # = g * (last_c + g)
    hidden_state = g * cell_state
    returns (cell_state, hidden_state)       # each [H, B] f32

Sharding: pure data parallel along the batch (column) axis B across 8
NeuronCores; the four [128,128] weights are replicated (pre-transposed on
host so they can feed the PE as lhsT directly).
"""

import sys
from contextlib import ExitStack

import numpy as np

for _p in ("/opt/trn_rl_repo", "/opt/pypackages"):
    if _p not in sys.path:
        sys.path.append(_p)

import concourse.bass as bass
import concourse.tile as tile
from concourse import bacc, mybir
from concourse import bass_utils

H = 128
S = 128
B = 131072
N_CORES = 8
B_CORE = B // N_CORES  # 16384 columns per core

N_TILE = 2048  # columns per SBUF tile (1 MiB per [128, N_TILE] f32 DMA)
MM_FREE = 512  # matmul free dim = one PSUM bank of f32

F32 = mybir.dt.float32

BIG_INPUTS = ("s_in", "s_out", "h_in", "h_out", "last_c")
WEIGHTS = ("w_in_input", "w_out_input", "u_in_input", "u_out_input")
WEIGHT_T_NAMES = tuple(w + "_T" for w in WEIGHTS)


F32R = mybir.dt.float32r


def emit_lstm_tile(ctx: ExitStack, tc: tile.TileContext, io: dict, b_core: int):
    """Per-core body.

    - loads issue on the Sync HWDGE ring, stores on the Scalar HWDGE ring
      (separate rings avoid head-of-line blocking of loads behind stores
      whose data isn't computed yet)
    - matmuls run as float32r (full-rate fp32 streaming, N=512 >= 256)
    - per-512-column chunk pipeline: PE (4 accum matmuls) -> ACT sigmoid
      -> GpSimd add -> DVE mul -> DVE mul; store issue is delayed by one
      chunk so the Scalar engine never stalls waiting for DVE results.
    """
    nc = tc.nc
    BLK = 2 * MM_FREE  # 1024-col elementwise/store block = 2 PSUM chunks
    n_tiles = b_core // N_TILE
    n_blocks = N_TILE // BLK

    wpool = ctx.enter_context(tc.tile_pool(name="weights", bufs=1))
    inpool = ctx.enter_context(tc.tile_pool(name="inp", bufs=3))
    gpool = ctx.enter_context(tc.tile_pool(name="gate", bufs=3))
    tpool = ctx.enter_context(tc.tile_pool(name="tmps", bufs=3))
    cpool = ctx.enter_context(tc.tile_pool(name="couts", bufs=3))
    hpool = ctx.enter_context(tc.tile_pool(name="houts", bufs=3))
    psum = ctx.enter_context(tc.tile_pool(name="psum", bufs=8, space="PSUM"))

    # weight loads go on the scalar (store) ring so the sync ring can start
    # streaming the big input tiles immediately
    wtiles = []
    for wname in WEIGHT_T_NAMES:
        wt = wpool.tile([S, H], F32R, name=f"w_{wname}")
        nc.scalar.dma_start(wt[:], io[wname][:].bitcast(F32R))
        wtiles.append(wt)
    w_i, w_o, u_i, u_o = wtiles

    pending_stores = None  # (c_chunk, h_chunk, dram_col_slice)

    def flush_stores():
        nonlocal pending_stores
        if pending_stores is not None:
            pc, ph, sl = pending_stores
            nc.scalar.dma_start(io["cell_state"][:, sl], pc[:])
            nc.scalar.dma_start(io["hidden_state"][:, sl], ph[:])
            pending_stores = None

    for i in range(n_tiles):
        ts = bass.ts(i, N_TILE)
        t_sin = inpool.tile([S, N_TILE], F32R, name="t_sin")
        nc.sync.dma_start(t_sin[:], io["s_in"][:, ts].bitcast(F32R))
        t_sout = inpool.tile([S, N_TILE], F32R, name="t_sout")
        nc.sync.dma_start(t_sout[:], io["s_out"][:, ts].bitcast(F32R))
        t_hin = inpool.tile([H, N_TILE], F32R, name="t_hin")
        nc.sync.dma_start(t_hin[:], io["h_in"][:, ts].bitcast(F32R))
        t_hout = inpool.tile([H, N_TILE], F32R, name="t_hout")
        nc.sync.dma_start(t_hout[:], io["h_out"][:, ts].bitcast(F32R))
        t_lc = inpool.tile([H, N_TILE], F32, name="t_lc")
        nc.sync.dma_start(t_lc[:], io["last_c"][:, ts])

        for b in range(n_blocks):
            g = gpool.tile([H, BLK], F32, name="g")
            for j in range(BLK // MM_FREE):
                js = bass.ts(b * 2 + j, MM_FREE)  # within the 2048 tile
                ps = psum.tile([H, MM_FREE], F32, name="ps")
                nc.tensor.matmul(ps[:], w_i[:], t_sin[:, js], start=True, stop=False)
                nc.tensor.matmul(ps[:], w_o[:], t_sout[:, js], start=False, stop=False)
                nc.tensor.matmul(ps[:], u_i[:], t_hin[:, js], start=False, stop=False)
                nc.tensor.matmul(ps[:], u_o[:], t_hout[:, js], start=False, stop=True)
                nc.scalar.activation(
                    g[:, bass.ts(j, MM_FREE)], ps[:],
                    mybir.ActivationFunctionType.Sigmoid,
                )
            flush_stores()  # previous block's c/h are ready by now

            # c = g * (last_c + g); h = g * c  -- all on DVE, back to back
            bs = bass.ts(b, BLK)
            tmp = tpool.tile([H, BLK], F32, name="tmp")
            nc.vector.tensor_add(tmp[:], g[:], t_lc[:, bs])
            c = cpool.tile([H, BLK], F32, name="c")
            nc.vector.tensor_mul(c[:], g[:], tmp[:])
            h = hpool.tile([H, BLK], F32, name="h")
            nc.vector.tensor_mul(h[:], g[:], c[:])
            pending_stores = (c, h, bass.ts(i * n_blocks + b, BLK))

    flush_stores()


def build_model(b_core: int = B_CORE, n_cores: int = N_CORES):
    nc = bacc.Bacc(
        "TRN2",
        target_bir_lowering=False,
        debug=False,
        enable_asserts=False,
        num_devices=n_cores,
    )
    io = {}
    for name in BIG_INPUTS:
        io[name] = nc.dram_tensor(name, [S, b_core], F32, kind="ExternalInput").ap()
    for name in WEIGHT_T_NAMES:
        io[name] = nc.dram_tensor(name, [S, H], F32, kind="ExternalInput").ap()
    io["cell_state"] = nc.dram_tensor(
        "cell_state", [H, b_core], F32, kind="ExternalOutput"
    ).ap()
    io["hidden_state"] = nc.dram_tensor(
        "hidden_state", [H, b_core], F32, kind="ExternalOutput"
    ).ap()

    with tile.TileContext(nc) as tc, ExitStack() as ctx:
        emit_lstm_tile(ctx, tc, io, b_core)
    nc.compile()
    return nc


_model_cache: dict = {}


def _get_model():
    if "nc" not in _model_cache:
        _model_cache["nc"] = build_model()
    return _model_cache["nc"]


def make_in_maps(inputs: dict, b_core: int = B_CORE, n_cores: int = N_CORES):
    weights_t = {
        wname + "_T": np.ascontiguousarray(np.asarray(inputs[wname]).T)
        for wname in WEIGHTS
    }
    in_maps = []
    for c in range(n_cores):
        sl = slice(c * b_core, (c + 1) * b_core)
        m = {
            name: np.ascontiguousarray(np.asarray(inputs[name])[:, sl])
            for name in BIG_INPUTS
        }
        m.update(weights_t)
        in_maps.append(m)
    return in_maps


def run_spmd(inputs: dict, trace: bool = False, **kwargs):
    nc = _get_model()
    in_maps = make_in_maps(inputs)
    res = bass_utils.run_bass_kernel_spmd(
        nc, in_maps, core_ids=list(range(N_CORES)), trace=trace, **kwargs
    )
    cell = np.concatenate(
        [res.results[c]["cell_state"] for c in range(N_CORES)], axis=1
    )
    hidden = np.concatenate(
        [res.results[c]["hidden_state"] for c in range(N_CORES)], axis=1
    )
    return (cell, hidden), res


def kernel(**inputs):
    outs, _ = run_spmd(inputs, trace=False)
    return outs


# revision 16
# speedup vs baseline: 1.1034x; 1.1034x over previous
"""Trainium2 Bass kernel for nn_CustomizeLSTMCell.

reference:
    pre = w_in_input @ s_in + w_out_input @ s_out + u_in_input @ h_in + u_out_input @ h_out
    g = sigmoid(pre)
    cell_state = g * last_c + g * g          # = g * (last_c + g)
    hidden_state = g * cell_state
    returns (cell_state, hidden_state)       # each [H, B] f32

Sharding: pure data parallel along the batch (column) axis B across 8
NeuronCores; the four [128,128] weights are replicated (pre-transposed on
host so they can feed the PE as lhsT directly).

Device-side layout tricks (all pack/unpack happens on host):
  - the four matmul operand tensors are interleaved per 2048-column tile
    into one DRAM tensor, so each tile needs ONE 4 MiB load instead of 4.
  - last_c stays a separate f32 tensor (keeps it out of the f32r path).
  - per 1024-column block, cell/hidden outputs are written side by side
    in one SBUF tile and leave via ONE 1 MiB store.
"""

import sys
from contextlib import ExitStack

import numpy as np

for _p in ("/opt/trn_rl_repo", "/opt/pypackages"):
    if _p not in sys.path:
        sys.path.append(_p)

import concourse.bass as bass
import concourse.tile as tile
from concourse import bacc, mybir
from concourse import bass_utils

H = 128
S = 128
B = 131072
N_CORES = 8
B_CORE = B // N_CORES  # 16384 columns per core

N_TILE = 2048          # columns per load tile
BLK = 1024             # columns per elementwise/store block
MM_FREE = 512          # matmul free dim = one PSUM bank of f32

F32 = mybir.dt.float32
F32R = mybir.dt.float32r

MM_INPUTS = ("s_in", "s_out", "h_in", "h_out")  # packed, matmul operands
WEIGHTS = ("w_in_input", "w_out_input", "u_in_input", "u_out_input")
WEIGHT_T_NAMES = tuple(w + "_T" for w in WEIGHTS)
N_MM = len(MM_INPUTS)


def tile_plan(b_core: int):
    """List of (col_offset, tile_cols). The final N_TILE columns are split
    into two half tiles so the endgame load->compute->store chain after the
    very last load is half as deep."""
    n_full = b_core // N_TILE - 1
    plan = [(i * N_TILE, N_TILE) for i in range(n_full)]
    base = n_full * N_TILE
    plan.append((base, N_TILE // 2))
    plan.append((base + N_TILE // 2, N_TILE // 2))
    return plan


def pack_mm_inputs(arrs, b_core: int):
    """[4][128, b_core] f32 -> [n_rowblocks*128, 4*N_TILE] tile-major: each
    tile from tile_plan() is a contiguous [s_in|s_out|h_in|h_out] block of
    width 4*tile_cols; consecutive tiles fill row-blocks left to right (the
    two half tiles share the last row-block)."""
    n_rb = b_core // N_TILE
    out = np.empty((n_rb * H, N_MM * N_TILE), dtype=np.float32)
    rb, col = 0, 0
    for off, tc in tile_plan(b_core):
        blk = np.concatenate([a[:, off : off + tc] for a in arrs], axis=1)
        out[rb * H : (rb + 1) * H, col : col + N_MM * tc] = blk
        col += N_MM * tc
        if col == N_MM * N_TILE:
            rb, col = rb + 1, 0
    return out


def pack_lc(arr, b_core: int):
    """[128, b_core] -> [n_rowblocks*128, N_TILE], same tile-major scheme."""
    n_rb = b_core // N_TILE
    out = np.empty((n_rb * H, N_TILE), dtype=np.float32)
    rb, col = 0, 0
    for off, tc in tile_plan(b_core):
        out[rb * H : (rb + 1) * H, col : col + tc] = arr[:, off : off + tc]
        col += tc
        if col == N_TILE:
            rb, col = rb + 1, 0
    return out


def unpack_outputs(packed, b_core: int):
    """[n_blocks*128, 2*BLK] block-major [c_b | h_b] -> (cell, hidden)."""
    n_blocks = b_core // BLK
    v = packed.reshape(n_blocks, H, 2, BLK)
    c = v[:, :, 0, :].transpose(1, 0, 2).reshape(H, b_core)
    h = v[:, :, 1, :].transpose(1, 0, 2).reshape(H, b_core)
    return np.ascontiguousarray(c), np.ascontiguousarray(h)


def emit_lstm_tile(ctx: ExitStack, tc: tile.TileContext, io: dict, b_core: int):
    """Per-core body.

    - loads issue on the Sync HWDGE ring, stores + weight loads on the
      Scalar HWDGE ring (separate rings avoid head-of-line blocking of
      loads behind stores whose data isn't computed yet)
    - matmuls run as float32r (full-rate fp32 streaming, N=512 >= 256)
    - per-512-column chunk: 4 accumulating matmuls -> PSUM, ACT sigmoid
      PSUM -> SBUF; per-1024-column block: 3 DVE ops back to back, then
      one packed c|h store whose issue is delayed by one block so the
      Scalar engine never stalls waiting on DVE results.
    """
    nc = tc.nc
    n_tiles = b_core // N_TILE
    n_blocks_per_tile = N_TILE // BLK

    wpool = ctx.enter_context(tc.tile_pool(name="weights", bufs=1))
    inpool = ctx.enter_context(tc.tile_pool(name="inp", bufs=3))
    lcpool = ctx.enter_context(tc.tile_pool(name="lc", bufs=3))
    gpool = ctx.enter_context(tc.tile_pool(name="gate", bufs=3))
    tpool = ctx.enter_context(tc.tile_pool(name="tmps", bufs=3))
    opool = ctx.enter_context(tc.tile_pool(name="chout", bufs=3))
    psum = ctx.enter_context(tc.tile_pool(name="psum", bufs=8, space="PSUM"))

    wtiles = []
    for wname in WEIGHT_T_NAMES:
        wt = wpool.tile([S, H], F32R, name=f"w_{wname}")
        nc.scalar.dma_start(wt[:], io[wname][:].bitcast(F32R))
        wtiles.append(wt)

    pending_store = None  # (ch_tile, dram_col_slice)

    def flush_store():
        nonlocal pending_store
        if pending_store is not None:
            ch, blk_idx = pending_store
            nc.scalar.dma_start(io["out_packed"][bass.ts(blk_idx, H), :], ch[:])
            pending_store = None

    rb, col_in, col_lc = 0, 0, 0
    for off, tcols in tile_plan(b_core):
        t_in = inpool.tile([S, N_MM * tcols], F32R, name="t_in")
        nc.sync.dma_start(
            t_in[:],
            io["in_packed"][
                rb * S : (rb + 1) * S, col_in : col_in + N_MM * tcols
            ].bitcast(F32R),
        )
        t_lc = lcpool.tile([H, tcols], F32, name="t_lc")
        nc.sync.dma_start(
            t_lc[:], io["last_c"][rb * H : (rb + 1) * H, col_lc : col_lc + tcols]
        )
        col_in += N_MM * tcols
        col_lc += tcols
        if col_lc == N_TILE:
            rb, col_in, col_lc = rb + 1, 0, 0
        # operand views inside the packed tile
        ops = [t_in[:, bass.ts(k, tcols)] for k in range(N_MM)]

        for b in range(tcols // BLK):
            g = gpool.tile([H, BLK], F32, name="g")
            for j in range(BLK // MM_FREE):
                js = bass.ts(b * (BLK // MM_FREE) + j, MM_FREE)
                ps = psum.tile([H, MM_FREE], F32, name="ps")
                for k in range(N_MM):
                    nc.tensor.matmul(
                        ps[:], wtiles[k][:], ops[k][:, js],
                        start=(k == 0), stop=(k == N_MM - 1),
                    )
                nc.scalar.activation(
                    g[:, bass.ts(j, MM_FREE)], ps[:],
                    mybir.ActivationFunctionType.Sigmoid,
                )
            flush_store()  # previous block's c|h are ready by now

            # c = g * (last_c + g); h = g * c  -- all on DVE, back to back
            tmp = tpool.tile([H, BLK], F32, name="tmp")
            nc.vector.tensor_add(tmp[:], g[:], t_lc[:, bass.ts(b, BLK)])
            ch = opool.tile([H, 2 * BLK], F32, name="ch")
            nc.vector.tensor_mul(ch[:, 0:BLK], g[:], tmp[:])
            nc.vector.tensor_mul(ch[:, BLK : 2 * BLK], g[:], ch[:, 0:BLK])
            pending_store = (ch, off // BLK + b)

    flush_store()


def build_model(b_core: int = B_CORE, n_cores: int = N_CORES):
    nc = bacc.Bacc(
        "TRN2",
        target_bir_lowering=False,
        debug=False,
        enable_asserts=False,
        num_devices=n_cores,
    )
    n_tiles = b_core // N_TILE
    n_blocks = b_core // BLK
    io = {}
    io["in_packed"] = nc.dram_tensor(
        "in_packed", [n_tiles * S, N_MM * N_TILE], F32, kind="ExternalInput"
    ).ap()
    io["last_c"] = nc.dram_tensor(
        "last_c", [n_tiles * H, N_TILE], F32, kind="ExternalInput"
    ).ap()
    for name in WEIGHT_T_NAMES:
        io[name] = nc.dram_tensor(name, [S, H], F32, kind="ExternalInput").ap()
    io["out_packed"] = nc.dram_tensor(
        "out_packed", [n_blocks * H, 2 * BLK], F32, kind="ExternalOutput"
    ).ap()

    with tile.TileContext(nc) as tc, ExitStack() as ctx:
        emit_lstm_tile(ctx, tc, io, b_core)
    nc.compile()
    return nc


_model_cache: dict = {}


def _get_model():
    if "nc" not in _model_cache:
        _model_cache["nc"] = build_model()
    return _model_cache["nc"]


def make_in_maps(inputs: dict, b_core: int = B_CORE, n_cores: int = N_CORES):
    weights_t = {
        wname + "_T": np.ascontiguousarray(np.asarray(inputs[wname], dtype=np.float32).T)
        for wname in WEIGHTS
    }
    big = {k: np.asarray(inputs[k], dtype=np.float32) for k in MM_INPUTS + ("last_c",)}
    in_maps = []
    for c in range(n_cores):
        sl = slice(c * b_core, (c + 1) * b_core)
        m = {
            "in_packed": pack_mm_inputs([big[k][:, sl] for k in MM_INPUTS], b_core),
            "last_c": pack_lc(big["last_c"][:, sl], b_core),
        }
        m.update(weights_t)
        in_maps.append(m)
    return in_maps


def run_spmd(inputs: dict, trace: bool = False, **kwargs):
    nc = _get_model()
    in_maps = make_in_maps(inputs)
    res = bass_utils.run_bass_kernel_spmd(
        nc, in_maps, core_ids=list(range(N_CORES)), trace=trace, **kwargs
    )
    cells, hiddens = [], []
    for c in range(N_CORES):
        cell, hidden = unpack_outputs(res.results[c]["out_packed"], B_CORE)
        cells.append(cell)
        hiddens.append(hidden)
    return (
        np.concatenate(cells, axis=1),
        np.concatenate(hiddens, axis=1),
    ), res


def kernel(**inputs):
    outs, _ = run_spmd(inputs, trace=False)
    return outs
